# revision 22
# baseline (speedup 1.0000x reference)
"""BERT self-attention on 8 TRN2 NeuronCores.

Problem: hidden_states [4, 2048, 1024], 16 heads x 64 dim, fp32.
Sharding: core c handles batch b = c//2 and head-group g = c%2
(8 heads = 512 embedding columns per core). Full inputs in, full
output out; slicing/transposition/fp16-cast of inputs happens
host-side here.

Per-core device kernel (v14, all matmul operands fp16):
  Projections: Q^T/K^T [e,s] and V [s,e] from X^T [h,s] (weights
    pre-transposed host-side); V gets a ones column per head
    (softmax-denominator trick). Emitted just-in-time, interleaved
    into the attention stream to fill PE gaps (ACT is the limiter).
  Attention per (head-pair et, q-chunk qc): for each k-tile kt:
    S^T[k, 2x512q] on PE (contract d=64), expS = exp(S^T*0.125 +
    mask_k) on ACT -> fp16 SBUF, then ctx accumulated TRANSPOSED:
    out[q=128, 65] += ex[:, 128q-block].T @ V'[k, 65] -- 65-row
    matmuls at fp16 rate, 8 accumulators packed 4-per-PSUM-bank.
    Epilogue: DVE reciprocal of col 64 + per-partition scale of
    cols 0:64 straight out of PSUM; one DMA per (et, qc).
"""

import os
from collections import OrderedDict

import numpy as np

import concourse.bass as bass
import concourse.tile as tile
from concourse import bacc, mybir
from concourse.bass_utils import run_bass_kernel_spmd

F32 = mybir.dt.float32
F16 = mybir.dt.float16
I16 = mybir.dt.int16

# fp16 Schraudolph exp on DVE: i16 = scores*SCH_MUL + (mask*LOG2E_1024 +
# SCH_BIAS), bitcast int16 -> fp16.  Offloads exp tiles from ACT (the
# bottleneck) to DVE; softmax renormalization cancels the correlated part
# of the ~3% sawtooth error.
LOG2E_1024 = 1.4426950408889634 * 1024.0
SCH_MUL = 0.125 * LOG2E_1024
SCH_BIAS = 15.0 * 1024.0 - 366000.0 / (1 << 13)  # fp32 const rescaled to fp16
# chunks (et*4+qc) >= DVE_CHUNK0 route these kt tiles to DVE
DVE_CHUNK0 = 4
def dve_kts(ci):
    if ci < 4:
        return ()
    if ci < 10:
        return (1, 4, 7, 10, 13)
    return (1, 3, 6, 9, 11, 14)




B, S, H = 4, 2048, 1024
NH, HD = 16, 64
NCORES = 8
E = 512          # embedding columns per core (8 heads)
NHL = 8          # heads per core
NKT = S // 128   # 16 k-tiles
NET = E // 128   # 4 e-tiles (head pairs)
NHT = H // 128   # 8 h-tiles
QW = 512         # per-head q-chunk width

_CACHE = {}

KERNEL_VERSION = "v14"  # bump to bust the neuron compile cache on changes

LAST_PROFILE = {}


def build_kernel(with_vbias=True):
    nc = bacc.Bacc("TRN2", target_bir_lowering=False, debug=False,
                   num_devices=NCORES)

    xt = nc.dram_tensor("xt", [H, S], F16, kind="ExternalInput").ap()
    wqt = nc.dram_tensor("wqt", [H, E], F16, kind="ExternalInput").ap()
    wkt = nc.dram_tensor("wkt", [H, E], F16, kind="ExternalInput").ap()
    wvt = nc.dram_tensor("wvt", [H, E], F16, kind="ExternalInput").ap()
    bq2 = nc.dram_tensor("bq2", [128, NET], F32, kind="ExternalInput").ap()
    bk2 = nc.dram_tensor("bk2", [128, NET], F32, kind="ExternalInput").ap()
    bv2 = nc.dram_tensor("bv2", [1, E], F16, kind="ExternalInput").ap()
    suffix = f"{KERNEL_VERSION}{'b' if with_vbias else ''}"
    mask2 = nc.dram_tensor(f"mask2_{suffix}", [128, NKT], F32,
                           kind="ExternalInput").ap()
    smask2 = nc.dram_tensor("smask2", [128, NKT], F32,
                            kind="ExternalInput").ap()
    out = nc.dram_tensor("out", [S, E], F32, kind="ExternalOutput").ap()

    Exp = mybir.ActivationFunctionType.Exp

    with tile.TileContext(nc) as tc:
        with (
            tc.tile_pool(name="persist", bufs=1) as persist,
            tc.tile_pool(name="small", bufs=1) as small,
        ):
            # persistent SBUF tensors (single producer each)
            qt_t = [[persist.tile([128, QW], F16, name=f"qt_{et}_{sc}")
                     for sc in range(4)] for et in range(NET)]
            kt_t = [[persist.tile([128, QW], F16, name=f"kt_{et}_{sc}")
                     for sc in range(4)] for et in range(NET)]
            vp_t = [persist.tile([128, NHL * 65], F16, name=f"vp_{gst}")
                    for gst in range(NKT)]
            # staged full X^T and weights: one DMA per 8-tile group
            xb = [persist.tile([128, NHT * QW], F16, name=f"xb_{sc}")
                  for sc in range(4)]
            wkb = persist.tile([128, NHT * E], F16, name="wkb")
            wqb = persist.tile([128, NHT * E], F16, name="wqb")
            wvb = persist.tile([128, NHT * E], F16, name="wvb")

            # big input DMAs (first-needed first)
            def big_dma(dst, src, w):
                nc.sync.dma_start(
                    dst.rearrange("p (h c) -> p h c", c=w),
                    src.rearrange("(h p) c -> p h c", p=128))

            big_dma(wkb[:, 0:4 * E], wkt[0:512, :], E)
            big_dma(xb[0][:, 0:4 * QW], xt[0:512, 0:QW], QW)
            big_dma(wqb[:, 0:4 * E], wqt[0:512, :], E)
            big_dma(wkb[:, 4 * E:], wkt[512:1024, :], E)
            big_dma(xb[0][:, 4 * QW:], xt[512:1024, 0:QW], QW)
            big_dma(wqb[:, 4 * E:], wqt[512:1024, :], E)
            bq_sb = small.tile([128, NET], F32)
            nc.sync.dma_start(bq_sb[:], bq2)
            bk_sb = small.tile([128, NET], F32)
            nc.sync.dma_start(bk_sb[:], bk2)
            mask_sb = small.tile([128, NKT], F32)
            nc.sync.dma_start(mask_sb[:], mask2)
            smask_sb = small.tile([128, NKT], F32)
            nc.sync.dma_start(smask_sb[:], smask2)
            big_dma(wvb[:], wvt, E)
            bv_sb = small.tile([1, E], F16)
            nc.sync.dma_start(bv_sb[:], bv2)
            for sc in range(1, 4):
                big_dma(xb[sc][:], xt[:, sc * QW:(sc + 1) * QW], QW)

            ones16 = small.tile([128, NHL], F16)
            nc.vector.memset(ones16[:], 1.0)
            ones_row = small.tile([1, 128], F16)
            nc.vector.memset(ones_row[:], 1.0)

            # ones columns of V' (denominator trick)
            for gst in range(NKT):
                vcols = vp_t[gst].rearrange("p (t c) -> p t c", c=65)
                nc.vector.tensor_copy(
                    vcols[:, :, 64:65],
                    ones16.rearrange("p (t c) -> p t c", c=1))

            with (
                tc.tile_pool(name="ssp", bufs=3, space="PSUM") as ssp,
                tc.tile_pool(name="ctxp", bufs=2, space="PSUM") as ctxp,
                tc.tile_pool(name="exp", bufs=11) as ex_pool,
                tc.tile_pool(name="obp", bufs=2) as ob_pool,
                tc.tile_pool(name="rcp", bufs=8) as rc_pool,
            ):
                # ---- just-in-time projection job machinery ----
                # each proj group = 8 accumulating matmuls + 1 DVE op,
                # emitted as single-instruction closures so they can be
                # interleaved into the attention stream (PE slack).
                pending = OrderedDict()   # key -> list of closures
                done = set()

                def kq_group(kind, et, sc):
                    wb = wkb if kind == "k" else wqb
                    b_sb = bk_sb if kind == "k" else bq_sb
                    dst = (kt_t if kind == "k" else qt_t)[et][sc]
                    cell = {}

                    def mk_mm(h):
                        def go():
                            if "p" not in cell:
                                cell["p"] = ssp.tile(
                                    [128, 1024], F32, tag="ss",
                                    name=f"pj{kind}_{et}_{sc}")
                            nc.tensor.matmul(
                                cell["p"][:, 0:QW],
                                wb[:, h * E + et * 128:h * E + et * 128 + 128],
                                xb[sc][:, h * QW:(h + 1) * QW],
                                start=(h == 0), stop=(h == NHT - 1))
                        return go

                    def bias():
                        nc.vector.tensor_scalar_add(
                            dst[:], cell["p"][:, 0:QW], b_sb[:, et:et + 1])
                    return [mk_mm(h) for h in range(NHT)] + [bias]

                def v_group(gst):
                    sc, st = gst // 4, gst % 4
                    cell = {}

                    def mk_mm(h):
                        def go():
                            if "p" not in cell:
                                cell["p"] = ssp.tile(
                                    [128, 1024], F32, tag="ss",
                                    name=f"pjv_{gst}")
                            nc.tensor.matmul(
                                cell["p"][:, 0:E],
                                xb[sc][:, h * QW + st * 128:
                                        h * QW + st * 128 + 128],
                                wvb[:, h * E:(h + 1) * E],
                                start=(h == 0),
                                stop=(not with_vbias and h == NHT - 1))
                        return go

                    jobs = [mk_mm(h) for h in range(NHT)]
                    if with_vbias:
                        def vb():
                            nc.tensor.matmul(cell["p"][:, 0:E], ones_row[:],
                                             bv_sb[:], start=False, stop=True)
                        jobs.append(vb)

                    def copy():
                        dstv = vp_t[gst].rearrange("p (t c) -> p t c", c=65)
                        nc.vector.tensor_copy(
                            dstv[:, :, 0:64],
                            cell["p"][:, 0:E].rearrange(
                                "p (t c) -> p t c", c=64))
                    jobs.append(copy)
                    return jobs

                def enqueue(key):
                    if key[0] == "v":
                        pending[key] = v_group(key[1])
                    else:
                        pending[key] = kq_group(*key)

                def require(key):
                    if key in done:
                        return
                    for job in pending.pop(key):
                        job()
                    done.add(key)

                def pop_group():
                    if not pending:
                        return
                    key = next(iter(pending))
                    require(key)

                def pop_half_group(n=5):
                    if not pending:
                        return
                    key = next(iter(pending))
                    jobs = pending[key]
                    for _ in range(min(n, len(jobs))):
                        jobs.pop(0)()
                    if not jobs:
                        del pending[key]
                        done.add(key)

                # queue in rough need-order
                enqueue(("k", 0, 0))
                enqueue(("q", 0, 0))
                for gst in range(4):
                    enqueue(("v", gst))
                for sc in range(1, 4):
                    enqueue(("k", 0, sc))
                    for gst in range(4 * sc, 4 * sc + 4):
                        enqueue(("v", gst))
                for qc in range(1, 4):
                    enqueue(("q", 0, qc))
                for et in range(1, NET):
                    for sc in range(4):
                        enqueue(("k", et, sc))
                    for qc in range(4):
                        enqueue(("q", et, qc))

                # eager prologue: minimum to start chunk (0, 0)
                require(("k", 0, 0))
                require(("q", 0, 0))

                # ---- attention chunks ----
                def chunk(et, qc):
                    ci = et * 4 + qc
                    kts = dve_kts(ci)
                    require(("q", et, qc))
                    qtt = qt_t[et][qc]
                    cbs = [ctxp.tile([128, 512], F32, tag="ctx",
                                     name=f"cb{half}_{et}_{qc}")
                           for half in range(2)]
                    subs = [cbs[i // 4][:, (i % 4) * 65:(i % 4) * 65 + 65]
                            for i in range(8)]
                    exs = [None] * NKT

                    def ctx_mms(kt):
                        for hl in range(2):
                            h = 2 * et + hl
                            for qb in range(4):
                                nc.tensor.matmul(
                                    subs[hl * 4 + qb],
                                    exs[kt][:, hl * QW + qb * 128:
                                            hl * QW + qb * 128 + 128],
                                    vp_t[kt][:, h * 65:h * 65 + 65],
                                    start=(kt == 0), stop=(kt == NKT - 1))

                    for kt in range(NKT):
                        if kt % 4 == 0:
                            require(("k", et, kt // 4))
                        require(("v", kt))
                        ktt = kt_t[et][kt // 4]
                        ko = (kt % 4) * 128
                        sps = ssp.tile([128, 2 * QW], F32, tag="ss",
                                       name=f"sps_{et}_{qc}_{kt}")
                        nc.tensor.matmul(
                            sps[:, 0:QW],
                            ktt[0:64, ko:ko + 128],
                            qtt[0:64, :], start=True, stop=True)
                        nc.tensor.matmul(
                            sps[:, QW:2 * QW],
                            ktt[64:128, ko:ko + 128],
                            qtt[64:128, :], start=True, stop=True)
                        ex = ex_pool.tile([128, 2 * QW], F16, tag="ex",
                                          name=f"ex_{et}_{qc}_{kt}")
                        if kt in kts:
                            nc.vector.tensor_scalar(
                                ex[:].bitcast(I16), sps[:],
                                SCH_MUL, smask_sb[:, kt:kt + 1],
                                mybir.AluOpType.mult, mybir.AluOpType.add)
                        else:
                            nc.scalar.activation(
                                ex[:], sps[:], Exp,
                                bias=mask_sb[:, kt:kt + 1], scale=0.125)
                        exs[kt] = ex
                        lag = 2 if ci == 15 else 8
                        if kt >= lag:
                            ctx_mms(kt - lag)  # deep software pipeline: PE
                        # prefetch next chunk's K/Q spread through this one
                        if qc == 3 and et < 3 and kt % 4 == 2:
                            require(("k", et + 1, kt // 4))
                        if kt == 13 and ci < 15:
                            nq = (et, qc + 1) if qc < 3 else (et + 1, 0)
                            require(("q",) + nq)
                        if kt % 2 == 1:       # never waits on exp directly
                            # near-atomic: a proj group holds its ssp slot
                            # only ~2 steps; ~1us bursts keep ACT fed
                            pop_half_group()
                            if ci in (1, 2):
                                pop_half_group()
                    for _t in range(NKT - lag, NKT):
                        ctx_mms(_t)

                    # epilogue: normalize straight out of PSUM, one DMA
                    obig = ob_pool.tile([128, 512], F32, tag="ob",
                                        name=f"ob_{et}_{qc}")
                    rcs = []
                    for hl in range(2):
                        rc4 = rc_pool.tile([128, 4], F32, tag="rc",
                                           name=f"rc_{et}_{qc}_{hl}")
                        nc.vector.reciprocal(
                            rc4[:], cbs[hl][:, 0:260].rearrange(
                                "p (t c) -> p t c", c=65)[:, :, 64:65])
                        rcs.append(rc4)
                    for hl in range(2):
                        for qb in range(4):
                            sub = subs[hl * 4 + qb]
                            eng = nc.vector if qb % 2 == hl else nc.gpsimd
                            eng.tensor_scalar_mul(
                                obig[:, qb * 128 + hl * 64:
                                     qb * 128 + hl * 64 + 64],
                                sub[:, 0:64], rcs[hl][:, qb:qb + 1])
                        if ci == 15 and hl == 0:
                            od = out[qc * QW:(qc + 1) * QW,
                                     et * 128:et * 128 + 64]
                            od = od.rearrange("(i p) c -> p i c", p=128)
                            nc.sync.dma_start(
                                od, obig.rearrange(
                                    "p (i c) -> p i c", c=128)[:, :, 0:64])
                    if ci == 15:
                        od = out[qc * QW:(qc + 1) * QW,
                                 et * 128 + 64:(et + 1) * 128]
                        od = od.rearrange("(i p) c -> p i c", p=128)
                        nc.sync.dma_start(
                            od, obig.rearrange(
                                "p (i c) -> p i c", c=128)[:, :, 64:128])
                    else:
                        od = out[qc * QW:(qc + 1) * QW,
                                 et * 128:(et + 1) * 128]
                        od = od.rearrange("(i p) c -> p i c", p=128)
                        nc.sync.dma_start(
                            od, obig.rearrange("p (i c) -> p i c", c=128))

                for et in range(NET):
                    for qc in range(4):
                        chunk(et, qc)

                # flush any leftover projection jobs (shouldn't happen)
                while pending:
                    pop_jobs(64)

    nc.compile()
    return nc


def build_in_maps(inputs, with_vbias=None):
    if with_vbias is None:
        with_vbias = bool(np.any(np.asarray(inputs["bv"], np.float32)))
    vb = "b" if with_vbias else ""
    hidden_states = np.asarray(inputs["hidden_states"], dtype=np.float32)
    attention_mask = np.asarray(inputs["attention_mask"], dtype=np.float32)
    Wq, bq = np.asarray(inputs["Wq"], np.float32), np.asarray(inputs["bq"], np.float32)
    Wk, bk = np.asarray(inputs["Wk"], np.float32), np.asarray(inputs["bk"], np.float32)
    Wv, bv = np.asarray(inputs["Wv"], np.float32), np.asarray(inputs["bv"], np.float32)

    xts = [np.ascontiguousarray(hidden_states[b].T).astype(np.float16)
           for b in range(B)]
    masks = [np.ascontiguousarray(attention_mask[b, 0, 0].reshape(NKT, 128).T)
             for b in range(B)]
    wg = []
    for g in range(2):
        rows = slice(g * E, (g + 1) * E)
        wg.append({
            "wqt": np.ascontiguousarray(Wq[rows].T).astype(np.float16),
            "wkt": np.ascontiguousarray(Wk[rows].T).astype(np.float16),
            "wvt": np.ascontiguousarray(Wv[rows].T).astype(np.float16),
            "bq2": np.ascontiguousarray(bq[rows].reshape(NET, 128).T),
            "bk2": np.ascontiguousarray(bk[rows].reshape(NET, 128).T),
            "bv2": np.ascontiguousarray(bv[rows].reshape(1, E)).astype(
                np.float16),
        })
    in_maps = []
    for c in range(NCORES):
        b, g = c // 2, c % 2
        in_maps.append({
            "xt": xts[b],
            f"mask2_{KERNEL_VERSION}{vb}": masks[b],
            "smask2": np.ascontiguousarray(
                masks[b] * np.float32(LOG2E_1024) + np.float32(SCH_BIAS)),
            **wg[g],
        })
    return in_maps


def kernel(hidden_states, attention_mask, Wq, bq, Wk, bk, Wv, bv):
    with_vbias = bool(np.any(np.asarray(bv, np.float32)))
    ckey = ("nc", with_vbias)
    if ckey not in _CACHE:
        _CACHE[ckey] = build_kernel(with_vbias)
    nc = _CACHE[ckey]

    in_maps = build_in_maps(dict(
        hidden_states=hidden_states, attention_mask=attention_mask,
        Wq=Wq, bq=bq, Wk=Wk, bk=bk, Wv=Wv, bv=bv))

    trace = bool(int(os.environ.get("BASS_KERNEL_TRACE", "0")))
    res = run_bass_kernel_spmd(nc, in_maps, core_ids=list(range(NCORES)),
                               trace=trace)
    LAST_PROFILE["exec_time_ns"] = res.exec_time_ns
    LAST_PROFILE["mean_exec_time_ns"] = res.mean_exec_time_ns
    if res.instructions_and_trace is not None:
        LAST_PROFILE["trace_path"] = res.instructions_and_trace[1]

    full = np.empty((B, S, H), dtype=np.float32)
    for c in range(NCORES):
        b, g = c // 2, c % 2
        full[b][:, g * E:(g + 1) * E] = res.results[c]["out"]
    return full
